# revision 1
# baseline (speedup 1.0000x reference)
"""Trainium2 Bass kernel for nn_DiracGraphConv (GNN edge-softmax message passing).

Strategy (8 NeuronCores, SPMD, no collectives):
  - Shard edges by DESTINATION node range: core k owns nodes
    [k*12500, (k+1)*12500) and processes exactly the edges whose row
    (destination) falls in that range. Segment-sums for a node happen
    entirely on its owner core, so per-core results are disjoint node
    slabs and the full output is a host-side concatenation.
  - Within a core, edges are bucketed by col//25000 into 4 groups so
    gather indices fit int16 (dma_gather/dma_scatter_add contract).
  - The core's z slab is L2-normalized once on device (zh table), so the
    per-edge cosine needs only num = zh[row]&middot;z[col] and |z[col]|:
    corr = num / max(|z_col|, eps). exp shift constant is |alpha|
    (softmax shift invariance; bias_edge cancels).
  - Per gather-chunk: dma_gather zh[row] (row-local slab) and zx[col]
    (combined [z | x] 512-byte rows), compute logits and exp on DVE/ACT
    (exp lands directly in the message's 65th column), then
    dma_scatter_add the 65-wide message [e * x[col], e] into a per-core
    DRAM accumulator.
  - HW dma_scatter_add races on duplicate indices within an instruction
    (and across concurrently-flying instructions) — verified on HW.
    Countermeasures:
    (a) the host deals each (core,group,row)'s edges round-robin across
        scatter sub-chunks, so every scatter instruction carries unique
        row indices (pad tokens all hit a junk row; races there are
        harmless);
    (b) scatter instructions rotate across n_accums accumulator buffers;
        scatters on the same buffer are WAW-serialized by Tile sems, so
        same-row descriptors from different instructions are never in
        flight together. Final phase sums the buffers.
  - Final phase (batched 4 node-tiles per iteration):
    out = (num / (denom + eps)) @ W^T + b via PE transpose + matmul with
    [W^T; b] and an appended ones-column.
"""

import sys

sys.path.insert(0, "/opt/trn_rl_repo")

import dataclasses
from dataclasses import dataclass

import numpy as np

from concourse import bacc, bass, mybir, tile
from concourse.library_config import mlp as MLP_LIB
from concourse.masks import make_identity

P = 128
F32 = mybir.dt.float32
I16 = mybir.dt.int16
EPS_DENOM = 1e-9
EPS_NORM = 1e-9


@dataclass(frozen=True)
class Cfg:
    n_cores: int = 8
    n_nodes: int = 100000
    d: int = 64
    nodes_per_core: int = 12500
    col_groups: int = 4
    col_group_size: int = 25000
    # SWDGE carveout fits <1024 descriptors per instruction (16B/desc in a
    # 16KB ring) — every dma_gather/dma_scatter_add must stay below that.
    tokens_per_group: int = 52224  # multiple of scatter_b (auto-grown if needed)
    gather_b: int = 768  # max tokens per gather/compute chunk (<1024 descs)
    scatter_b: int = 768  # tokens per scatter instruction (unique rows)
    n_accums: int = 4

    @property
    def acc_rows(self) -> int:
        # accumulator rows: nodes_per_core real + 1 junk row, padded to 128
        return ((self.nodes_per_core + 1 + P - 1) // P) * P

    @property
    def junk_row(self) -> int:
        return self.nodes_per_core

    @property
    def n_scatter_chunks(self) -> int:
        return self.tokens_per_group // self.scatter_b

    def gather_chunks(self):
        sizes = []
        t = self.tokens_per_group
        while t > 0:
            b = min(self.gather_b, t)
            sizes.append(b)
            t -= b
        assert all(s % self.scatter_b == 0 for s in sizes)
        return sizes


FULL = Cfg()


def build_program(cfg: Cfg, alpha: float):
    """One SPMD program for all cores. Inputs (per core):
    zrow [acc_rows, d] f32   core's raw z slab
    zx   [col_groups*col_group_size, 2d] f32   full [z | x] table
    ridx [col_groups, 128, tokens_per_group//16] i16
    cidx [col_groups, 128, tokens_per_group//16] i16
    wb   [d+1, d] f32  ([W^T; b])
    Output: out [acc_rows, d] f32 (rows >= nodes_per_core are garbage)
    """
    D = cfg.d
    DD = 2 * D
    TG16 = cfg.tokens_per_group // 16
    SB = cfg.scatter_b

    nc = bacc.Bacc(
        "TRN2", target_bir_lowering=False, debug=False, num_swdge_queues=1
    )

    zrow = nc.dram_tensor("zrow", [cfg.acc_rows, D], F32, kind="ExternalInput").ap()
    zxg = [
        nc.dram_tensor(f"zx{g}", [cfg.col_group_size, DD], F32, kind="ExternalInput").ap()
        for g in range(cfg.col_groups)
    ]
    ridx = nc.dram_tensor(
        "ridx", [cfg.col_groups, P, TG16], I16, kind="ExternalInput"
    ).ap()
    cidx = nc.dram_tensor(
        "cidx", [cfg.col_groups, P, TG16], I16, kind="ExternalInput"
    ).ap()
    wb = nc.dram_tensor("wb", [D + 1, D], F32, kind="ExternalInput").ap()
    out = nc.dram_tensor("out", [cfg.acc_rows, D], F32, kind="ExternalOutput").ap()

    zh = nc.dram_tensor("zh", [cfg.acc_rows, D], F32).ap()
    accums = [
        nc.dram_tensor(f"accum{s}", [cfg.acc_rows, DD], F32).ap()
        for s in range(cfg.n_accums)
    ]

    with tile.TileContext(nc) as tc:
        with (
            tc.tile_pool(name="const", bufs=1) as cpool,
            tc.tile_pool(name="idx", bufs=3) as ipool,
            tc.tile_pool(name="gath", bufs=2) as gpool,
            tc.tile_pool(name="work", bufs=2) as wpool,
            tc.tile_pool(name="smal", bufs=3) as spool,
            tc.tile_pool(name="fin", bufs=2) as fpool,
            tc.tile_pool(name="psum", bufs=2, space="PSUM") as ppool,
        ):
            # ---- constants ----
            nc.gpsimd.load_library(MLP_LIB)
            cb = cpool.tile([P, 1], F32, tag="cb")
            nc.vector.memset(cb[:], -abs(float(alpha)))
            ident = cpool.tile([P, P], F32, tag="ident")
            make_identity(nc, ident[:])
            wbs = cpool.tile([D + 1, D], F32, tag="wbs")
            nc.sync.dma_start(out=wbs[:], in_=wb[:, :])

            # ---- normalize the row slab: zh = zrow / max(|zrow|, eps) ----
            r0 = 0
            while r0 < cfg.acc_rows:
                j = min(8, (cfg.acc_rows - r0) // P)
                rows = slice(r0, r0 + j * P)
                zt_in = gpool.tile([P, 8, D], F32, tag="zi")
                nc.sync.dma_start(
                    out=zt_in[:, :j, :],
                    in_=zrow[rows, :].rearrange("(p a) d -> p a d", p=P),
                )
                sq = wpool.tile([P, 8, D], F32, tag="prod")
                nc.vector.tensor_tensor(
                    out=sq[:, :j, :], in0=zt_in[:, :j, :], in1=zt_in[:, :j, :],
                    op=mybir.AluOpType.mult,
                )
                ns = spool.tile([P, 8], F32, tag="ns")
                nc.vector.tensor_reduce(
                    out=ns[:, :j], in_=sq[:, :j, :], axis=mybir.AxisListType.X,
                    op=mybir.AluOpType.add,
                )
                nc.vector.tensor_scalar_max(ns[:, :j], ns[:, :j], 1e-18)
                nc.scalar.sqrt(out=ns[:, :j], in_=ns[:, :j])
                rr = spool.tile([P, 8], F32, tag="nr")
                nc.vector.reciprocal(out=rr[:, :j], in_=ns[:, :j])
                zo = gpool.tile([P, 8, D], F32, tag="gj")
                nc.vector.tensor_tensor(
                    out=zo[:, :j, :], in0=zt_in[:, :j, :],
                    in1=rr[:, :j].to_broadcast([P, j, D]), op=mybir.AluOpType.mult,
                )
                nc.sync.dma_start(
                    out=zh[rows, :].rearrange("(p a) d -> p a d", p=P),
                    in_=zo[:, :j, :],
                )
                r0 += j * P

            # ---- zero the accumulators ----
            acc_t = cfg.acc_rows // P
            zt = cpool.tile([P, 8 * DD], F32, tag="zt")
            nc.vector.memset(zt[:], 0.0)
            for acc in accums:
                acc_v = acc.rearrange("(t p) d -> p t d", p=P)
                for t0 in range(0, acc_t, 8):
                    nt = min(8, acc_t - t0)
                    nc.sync.dma_start(
                        out=acc_v[:, t0 : t0 + nt, :],
                        in_=zt[:, : nt * DD].rearrange("p (t d) -> p t d", d=DD),
                    )

            # ---- edge phase ----
            sc_counter = 0
            for g in range(cfg.col_groups):
                zx_win = zxg[g][:, :]
                c0 = 0
                for b in cfg.gather_chunks():
                    nb = b // P
                    s16 = slice(c0 // 16, (c0 + b) // 16)
                    rt = ipool.tile([P, cfg.gather_b // 16], I16, tag="rt")
                    ct = ipool.tile([P, cfg.gather_b // 16], I16, tag="ct")
                    nc.sync.dma_start(out=rt[:, : b // 16], in_=ridx[g, :, s16])
                    nc.sync.dma_start(out=ct[:, : b // 16], in_=cidx[g, :, s16])

                    zi = gpool.tile([P, cfg.gather_b // P, D], F32, tag="zi")
                    gj = gpool.tile([P, cfg.gather_b // P, DD], F32, tag="gj")
                    nc.gpsimd.dma_gather(
                        zi[:, :nb, :], zh[:, :], rt[:, : b // 16], b, b, D,
                        queue_num=0,
                    )
                    nc.gpsimd.dma_gather(
                        gj[:, :nb, :], zx_win, ct[:, : b // 16], b, b, DD,
                        queue_num=0,
                    )
                    gjz = gj[:, :nb, 0:D]
                    gjx = gj[:, :nb, D:DD]

                    prod = wpool.tile([P, cfg.gather_b // P, D], F32, tag="prod")
                    num = spool.tile([P, cfg.gather_b // P], F32, tag="num")
                    nj = spool.tile([P, cfg.gather_b // P], F32, tag="nj")
                    nc.vector.tensor_tensor(
                        out=prod[:, :nb, :], in0=zi[:, :nb, :], in1=gjz,
                        op=mybir.AluOpType.mult,
                    )
                    nc.vector.tensor_reduce(
                        out=num[:, :nb], in_=prod[:, :nb, :],
                        axis=mybir.AxisListType.X, op=mybir.AluOpType.add,
                    )
                    nc.vector.tensor_tensor(
                        out=prod[:, :nb, :], in0=gjz, in1=gjz,
                        op=mybir.AluOpType.mult,
                    )
                    nc.vector.tensor_reduce(
                        out=nj[:, :nb], in_=prod[:, :nb, :],
                        axis=mybir.AxisListType.X, op=mybir.AluOpType.add,
                    )
                    nc.vector.tensor_scalar_max(nj[:, :nb], nj[:, :nb], 1e-18)
                    nc.scalar.sqrt(out=nj[:, :nb], in_=nj[:, :nb])
                    rcp = spool.tile([P, cfg.gather_b // P], F32, tag="rcp")
                    nc.vector.reciprocal(out=rcp[:, :nb], in_=nj[:, :nb])
                    corr = spool.tile([P, cfg.gather_b // P], F32, tag="corr")
                    nc.vector.tensor_tensor(
                        out=corr[:, :nb], in0=num[:, :nb], in1=rcp[:, :nb],
                        op=mybir.AluOpType.mult,
                    )
                    msg = wpool.tile([P, cfg.gather_b // P, D + 1], F32, tag="msg")
                    # exp lands directly in the message's denom column
                    nc.scalar.activation(
                        out=msg[:, :nb, D : D + 1], in_=corr[:, :nb],
                        func=mybir.ActivationFunctionType.Exp,
                        bias=cb[:], scale=float(alpha),
                    )
                    nc.vector.tensor_tensor(
                        out=msg[:, :nb, 0:D], in0=gjx,
                        in1=msg[:, :nb, D : D + 1].to_broadcast([P, nb, D]),
                        op=mybir.AluOpType.mult,
                    )
                    # scatter sub-chunks: unique rows within each; rotate accums
                    for s in range(b // SB):
                        ai = sc_counter % cfg.n_accums
                        acc = accums[ai]
                        sc_counter += 1
                        msub = msg[:, s * (SB // P) : (s + 1) * (SB // P), :]
                        rsub = rt[:, s * (SB // 16) : (s + 1) * (SB // 16)]
                        nc.gpsimd.dma_scatter_add(
                            acc[:, 0 : D + 1], msub, rsub, SB, SB, D + 1,
                            elem_step=DD, queue_num=0,
                        )
                    c0 += b

            # ---- final phase: out = (num/(den+eps)) @ W^T + b, 4 tiles/iter ----
            out_v = out.rearrange("(t p) d -> p t d", p=P)
            acc_vs = [a.rearrange("(t p) d -> p t d", p=P) for a in accums]
            for t0 in range(0, acc_t, 4):
                nt = min(4, acc_t - t0)
                a = gpool.tile([P, 4, D + 1], F32, tag="zi")
                nc.sync.dma_start(
                    out=a[:, :nt, :], in_=acc_vs[0][:, t0 : t0 + nt, 0 : D + 1]
                )
                for s in range(1, cfg.n_accums):
                    a2 = gpool.tile([P, 4, D + 1], F32, tag="gj")
                    nc.sync.dma_start(
                        out=a2[:, :nt, :], in_=acc_vs[s][:, t0 : t0 + nt, 0 : D + 1]
                    )
                    nc.vector.tensor_tensor(
                        out=a[:, :nt, :], in0=a[:, :nt, :], in1=a2[:, :nt, :],
                        op=mybir.AluOpType.add,
                    )
                dplus = spool.tile([P, 4], F32, tag="dplus")
                nc.vector.tensor_scalar_add(
                    dplus[:, :nt], a[:, :nt, D : D + 1], EPS_DENOM
                )
                rr = spool.tile([P, 4], F32, tag="rr")
                nc.vector.reciprocal(out=rr[:, :nt], in_=dplus[:, :nt])
                m = wpool.tile([P, 4, D + 1], F32, tag="prod")
                nc.vector.tensor_tensor(
                    out=m[:, :nt, 0:D], in0=a[:, :nt, 0:D],
                    in1=rr[:, :nt].to_broadcast([P, nt, D]),
                    op=mybir.AluOpType.mult,
                )
                nc.vector.memset(m[:, :nt, D : D + 1], 1.0)
                o = wpool.tile([P, 4, D], F32, tag="msg")
                for i in range(nt):
                    tp = ppool.tile([D + 1, P], F32, tag="tp", space="PSUM")
                    nc.tensor.transpose(
                        out=tp[:], in_=m[:, i, :], identity=ident[:]
                    )
                    lhs = fpool.tile([D + 1, P], F32, tag="lhs")
                    nc.vector.tensor_copy(out=lhs[:], in_=tp[:])
                    y = ppool.tile([P, D], F32, tag="y", space="PSUM")
                    nc.tensor.matmul(
                        out=y[:], lhsT=lhs[:], rhs=wbs[:], start=True, stop=True
                    )
                    nc.scalar.copy(out=o[:, i, :], in_=y[:])
                nc.sync.dma_start(out=out_v[:, t0 : t0 + nt, :], in_=o[:, :nt, :])

    nc.compile()
    return nc


def _wrap16(a: np.ndarray) -> np.ndarray:
    # token i -> partition i%16, col i//16; replicated 8x to 128 partitions
    w = a.reshape(-1, 16).T
    return np.ascontiguousarray(np.tile(w, (8, 1)))


def shard_inputs(cfg: Cfg, x, z, edge_index):
    """Bucket edges by (owner core, col group); deal each row's edges across
    scatter sub-chunks so each scatter instruction has unique rows."""
    D = cfg.d
    row = np.asarray(edge_index[0], dtype=np.int64)
    col = np.asarray(edge_index[1], dtype=np.int64)
    E = row.shape[0]
    core = row // cfg.nodes_per_core
    grp = col // cfg.col_group_size
    bucket = core * cfg.col_groups + grp
    n_bins = cfg.n_cores * cfg.col_groups

    # rank of each edge within its (bucket, row) group
    o = np.lexsort((row, bucket))
    bs, rs = bucket[o], row[o]
    new = np.r_[True, (bs[1:] != bs[:-1]) | (rs[1:] != rs[:-1])]
    gid = np.cumsum(new) - 1
    pos = np.arange(E)
    firstpos = pos[new]
    rank = pos - firstpos[gid]
    maxmult = int(rank.max()) + 1 if E else 1

    # per-(bucket,row) group sizes and within-bucket exclusive cumsum: row r's
    # edges go to chunks (start_r + rank) % n — balanced to +-1 per bucket and
    # unique within each chunk (mult <= n_chunks)
    mult = np.bincount(gid)
    g_bucket = bs[new]
    g_cum = np.concatenate([[0], np.cumsum(mult)[:-1]])
    g_new_bucket = np.r_[True, g_bucket[1:] != g_bucket[:-1]]
    bucket_base = np.maximum.accumulate(np.where(g_new_bucket, g_cum, 0))
    g_start = g_cum - bucket_base
    bucket_counts = np.bincount(bs, minlength=n_bins)
    maxcount = int(bucket_counts.max()) if E else 1

    n_chunks = max(
        (maxcount + cfg.scatter_b - 1) // cfg.scatter_b, maxmult, 1
    )
    while True:
        chunkid = (g_start[gid] + rank) % n_chunks
        cc = np.bincount(bs * n_chunks + chunkid, minlength=n_bins * n_chunks)
        if maxmult <= n_chunks and cc.max() <= cfg.scatter_b:
            break
        n_chunks += 1
    eff = dataclasses.replace(cfg, tokens_per_group=n_chunks * cfg.scatter_b)

    # slot within (bucket, chunk)
    o2 = np.lexsort((chunkid, bs))
    b2, c2 = bs[o2], chunkid[o2]
    new2 = np.r_[True, (b2[1:] != b2[:-1]) | (c2[1:] != c2[:-1])]
    gid2 = np.cumsum(new2) - 1
    firstpos2 = pos[new2]
    rank2 = pos - firstpos2[gid2]
    tokpos = c2 * cfg.scatter_b + rank2

    TG = eff.tokens_per_group
    rl_all = np.full((n_bins, TG), eff.junk_row, np.int16)
    cl_all = np.zeros((n_bins, TG), np.int16)
    edge_sorted = o[o2]  # original edge ids in placement order
    flat = b2 * TG + tokpos
    rl_all.reshape(-1)[flat] = (row[edge_sorted] % cfg.nodes_per_core).astype(np.int16)
    cl_all.reshape(-1)[flat] = (col[edge_sorted] % cfg.col_group_size).astype(np.int16)

    zx = np.concatenate(
        [np.asarray(z, np.float32), np.asarray(x, np.float32)], axis=1
    )
    zx_groups = [
        np.ascontiguousarray(zx[g * cfg.col_group_size : (g + 1) * cfg.col_group_size])
        for g in range(cfg.col_groups)
    ]
    zpad = np.zeros((cfg.n_cores * cfg.nodes_per_core + cfg.acc_rows, D), np.float32)
    zpad[: cfg.n_nodes] = np.asarray(z, np.float32)

    in_maps = []
    for k in range(cfg.n_cores):
        ridx_g = np.stack(
            [_wrap16(rl_all[k * cfg.col_groups + g]) for g in range(cfg.col_groups)]
        )
        cidx_g = np.stack(
            [_wrap16(cl_all[k * cfg.col_groups + g]) for g in range(cfg.col_groups)]
        )
        in_maps.append(
            {
                "zrow": np.ascontiguousarray(
                    zpad[k * cfg.nodes_per_core : k * cfg.nodes_per_core + cfg.acc_rows]
                ),
                **{f"zx{g}": zx_groups[g] for g in range(cfg.col_groups)},
                "ridx": ridx_g,
                "cidx": cidx_g,
            }
        )
    return in_maps, eff


def run(cfg: Cfg, x, edge_index, z, W, b, alpha, bias_edge, trace=False):
    from concourse.bass_utils import run_bass_kernel_spmd

    in_maps, eff = shard_inputs(cfg, x, z, edge_index)
    wb = np.ascontiguousarray(
        np.concatenate(
            [np.asarray(W, np.float32).T, np.asarray(b, np.float32)[None, :]], axis=0
        )
    )
    for m in in_maps:
        m["wb"] = wb
    nc = build_program(eff, float(np.asarray(alpha)))
    core_ids = list(range(eff.n_cores))
    res = run_bass_kernel_spmd(nc, in_maps, core_ids, trace=trace)
    out = np.concatenate(
        [res.results[k]["out"][: eff.nodes_per_core] for k in core_ids], axis=0
    )[: eff.n_nodes]
    return out.astype(np.float32), res


def kernel(**inputs) -> np.ndarray:
    out, _ = run(
        FULL,
        inputs["x"],
        inputs["edge_index"],
        inputs["z"],
        inputs["W"],
        inputs["b"],
        inputs["alpha"],
        inputs["bias_edge"],
    )
    return out



# revision 5
# speedup vs baseline: 2.0043x; 2.0043x over previous
"""Trainium2 Bass kernel for nn_DiracGraphConv (GNN edge-softmax message passing).

V4 design (8 NeuronCores, SPMD, no collectives):
  - Shard edges by destination-node slab: core k owns rows
    [k*12500, (k+1)*12500). Per-core output slabs are disjoint; the full
    output is a host-side concatenation (row un-permute).
  - ONE SWDGE descriptor per edge (the previous kernel used three):
    a transposed dma_gather of 256B rows from a bf16 [zh | x] node table
    (zh = z/||z|| precomputed host-side), giving zh_col^T / x_col^T
    directly in SBUF. gpsimd descriptor generation (~7ns/desc, the hard
    bottleneck of this problem) is therefore minimized.
  - The segment-sum (edge softmax numerator/denominator) is done on the
    PE with a mask trick instead of dma_scatter_add:
      * rows are packed into NWIN windows of 64 rows (host bin-packing,
        balanced so every (window, col-group) cell holds <= 256 edges =
        2 subchunks of 128 -- a compile-time uniform schedule).
      * M2[e, r] = zh_col[e] . zh_win[r] via matmul (lhsT = gathered
        zh^T slice, rhs = per-window zh_win^T kept resident in SBUF).
      * masked[e, r] = (rowloc[e] == r) * exp(alpha*M2 - |alpha|); the
        exp shift is valid by softmax shift-invariance.
      * PSUM [64, 65] += masked^T @ [x_col | 1] accumulates messages and
        denominator per window; drained into an SBUF accumulator.
  - x_col in natural layout comes from dma_start_transpose (XBAR maps
    token t -> (partition t%128, slot t//128), matching the gather).
  - Final phase: out = (msgs/(denom+eps)) @ W^T + b via PE transpose +
    matmul with [W^T; b], from SBUF, then one DMA out per core.
"""

import sys

sys.path.insert(0, "/opt/trn_rl_repo")

from dataclasses import dataclass

import numpy as np
import ml_dtypes

from concourse import bacc, bass, mybir, tile
from concourse.library_config import mlp as MLP_LIB
from concourse.masks import make_identity

P = 128
F32 = mybir.dt.float32
BF16 = mybir.dt.bfloat16
I16 = mybir.dt.int16
EPS_DENOM = 1e-9

N_NODES = 100000
N_EDGES = 1600000
D = 64
DD = 128           # [zh | x] row elems (bf16) = 256B
N_CORES = 8
NPC = 12500        # nodes per core
NGRP = 4           # col groups (int16 gather index limit)
GRP = 25000        # nodes per col group
W = 64             # rows per window
CELL = 256         # max edges per (window, group) cell = 2 subchunks
STILE = 14         # slots per super-tile (2 gather instrs of 896)
GB = 896           # tokens per gather instruction


@dataclass(frozen=True)
class Cfg:
    nwin: int = 210  # windows per core; nwin % 7 == 0

    @property
    def nslotg(self) -> int:
        return self.nwin * 2          # subchunk slots per group stream

    @property
    def ntokg(self) -> int:
        return self.nslotg * P        # tokens per group stream

    @property
    def ntok(self) -> int:
        return self.ntokg * NGRP

    @property
    def acc_rows(self) -> int:
        return self.nwin * W


def build_program(cfg: Cfg, alpha: float):
    nwin = cfg.nwin
    nslotg = cfg.nslotg
    n_stile = nslotg // STILE

    nc = bacc.Bacc(
        "TRN2", target_bir_lowering=False, debug=False, num_swdge_queues=1
    )

    tabs = [
        nc.dram_tensor(f"tab{g}", [GRP, DD], BF16, kind="ExternalInput").ap()
        for g in range(NGRP)
    ]
    zhw = nc.dram_tensor("zhw", [D, nwin, W], BF16, kind="ExternalInput").ap()
    cidx = nc.dram_tensor(
        "cidx", [P, cfg.ntok // 16], I16, kind="ExternalInput"
    ).ap()
    rowloc = nc.dram_tensor(
        "rowloc", [P, cfg.ntok // P], BF16, kind="ExternalInput"
    ).ap()
    iota = nc.dram_tensor("iota", [P, W], BF16, kind="ExternalInput").ap()
    wb = nc.dram_tensor("wb", [D + 1, D], F32, kind="ExternalInput").ap()
    out = nc.dram_tensor("out", [cfg.acc_rows, D], F32, kind="ExternalOutput").ap()

    with tile.TileContext(nc) as tc:
        with (
            tc.tile_pool(name="const", bufs=1) as cpool,
            tc.tile_pool(name="gath", bufs=3) as gpool,
            tc.tile_pool(name="work", bufs=3) as wpool,
            tc.tile_pool(name="fin", bufs=2) as fpool,
            tc.tile_pool(name="mpsum", bufs=2, space="PSUM") as mpool,
            tc.tile_pool(name="cpsum", bufs=2, space="PSUM") as clpool,
            tc.tile_pool(name="fpsum", bufs=2, space="PSUM") as fppool,
        ):
            nc.gpsimd.load_library(MLP_LIB)
            # ---- resident SBUF state ----
            zhwt = cpool.tile([D, nwin, W], BF16, tag="zhwt")
            nc.sync.dma_start(out=zhwt[:], in_=zhw[:, :, :])
            cix = cpool.tile([P, cfg.ntok // 16], I16, tag="cix")
            nc.sync.dma_start(out=cix[:], in_=cidx[:, :])
            rlt = cpool.tile([P, cfg.ntok // P], BF16, tag="rlt")
            nc.sync.dma_start(out=rlt[:], in_=rowloc[:, :])
            iot = cpool.tile([P, 1, W], BF16, tag="iot")
            nc.sync.dma_start(out=iot[:, 0, :], in_=iota[:, :])
            wbs = cpool.tile([D + 1, D], F32, tag="wbs")
            nc.sync.dma_start(out=wbs[:], in_=wb[:, :])
            ident = cpool.tile([P, P], F32, tag="ident")
            make_identity(nc, ident[:])
            cb = cpool.tile([P, 1], F32, tag="cb")
            nc.vector.memset(cb[:], -abs(float(alpha)))
            acc = cpool.tile([D, nwin, D + 1], F32, tag="acc")
            nc.vector.memset(acc[:], 0.0)

            # ---- edge phase ----
            for g in range(NGRP):
                tab_g = tabs[g][:, :]
                for st in range(n_stile):
                    s0 = st * STILE                     # slot in group stream
                    tok0 = g * cfg.ntokg + (s0 * P)     # global token base
                    # two 896-token transposed gathers -> [128, 1, 1792]
                    gt = gpool.tile([P, 1, STILE * P], BF16, tag="gt")
                    for h in range(2):
                        nc.gpsimd.dma_gather(
                            gt[:, :, h * GB : (h + 1) * GB],
                            tab_g,
                            cix[:, (tok0 + h * GB) // 16 : (tok0 + (h + 1) * GB) // 16],
                            GB, GB, DD, transpose=True, queue_num=0,
                        )
                    # x natural: [64, 1792] -> [128, 14, 64]
                    xn = wpool.tile([P, STILE, D], BF16, tag="xn")
                    nc.sync.dma_start_transpose(out=xn[:], in_=gt[64:128, 0, :])
                    # aug copy [x | 1]
                    xa = wpool.tile([P, STILE, D + 1], BF16, tag="xa")
                    nc.vector.memset(xa[:, :, D : D + 1], 1.0)
                    nc.vector.tensor_copy(out=xa[:, :, 0:D], in_=xn[:])
                    # row-eq mask for the super-tile
                    rl = rlt[:, tok0 // P : tok0 // P + STILE]
                    eqm = wpool.tile([P, STILE, W], BF16, tag="eqm")
                    nc.vector.tensor_tensor(
                        out=eqm[:],
                        in0=rl.to_broadcast([P, STILE, W]),
                        in1=iot[:].to_broadcast([P, STILE, W]),
                        op=mybir.AluOpType.is_equal,
                    )
                    # M2 per subchunk; PSUM halves (7 slots = 1792B/bank)
                    ex = wpool.tile([P, STILE, W], BF16, tag="ex")
                    for h in range(2):
                        m2 = mpool.tile([P, STILE // 2, W], F32, tag="m2",
                                        space="PSUM")
                        for si in range(STILE // 2):
                            s = h * (STILE // 2) + si
                            w_id = (s0 + s) // 2
                            nc.tensor.matmul(
                                out=m2[:, si, :],
                                lhsT=gt[0:D, 0, s * P : (s + 1) * P],
                                rhs=zhwt[:, w_id, :],
                                start=True, stop=True,
                            )
                        nc.scalar.activation(
                            out=ex[:, h * (STILE // 2) : (h + 1) * (STILE // 2), :],
                            in_=m2[:],
                            func=mybir.ActivationFunctionType.Exp,
                            bias=cb[:], scale=float(alpha),
                        )
                    nc.vector.tensor_tensor(
                        out=ex[:], in0=ex[:], in1=eqm[:],
                        op=mybir.AluOpType.mult,
                    )
                    # aggregate: 7 cells x 2 subchunks
                    for c in range(STILE // 2):
                        w_id = (s0 + 2 * c) // 2
                        cell = clpool.tile([W, D + 1], F32, tag="cell",
                                           space="PSUM")
                        for j in range(2):
                            s = 2 * c + j
                            nc.tensor.matmul(
                                out=cell[:],
                                lhsT=ex[:, s, :],
                                rhs=xa[:, s, :],
                                start=(j == 0), stop=(j == 1),
                            )
                        nc.vector.tensor_tensor(
                            out=acc[:, w_id, :], in0=acc[:, w_id, :],
                            in1=cell[:], op=mybir.AluOpType.add,
                        )

            # ---- final phase: out = (msgs/(denom+eps)) @ W^T + b ----
            # repack [64, nwin, 65] -> [128, nwin//2, 65] via SBUF->SBUF DMA
            acc2 = cpool.tile([P, nwin // 2, D + 1], F32, tag="acc2")
            nc.sync.dma_start(out=acc2[0:D, :, :], in_=acc[:, 0::2, :])
            nc.sync.dma_start(out=acc2[D:P, :, :], in_=acc[:, 1::2, :])
            out_v = out.rearrange("(t p) d -> p t d", p=P)
            nt_all = nwin // 2
            for t0 in range(0, nt_all, 4):
                nt = min(4, nt_all - t0)
                a = fpool.tile([P, 4, D + 1], F32, tag="fa")
                dplus = fpool.tile([P, 4], F32, tag="dplus")
                nc.vector.tensor_scalar_add(
                    dplus[:, :nt], acc2[:, t0 : t0 + nt, D : D + 1], EPS_DENOM
                )
                rr = fpool.tile([P, 4], F32, tag="rr")
                nc.vector.reciprocal(out=rr[:, :nt], in_=dplus[:, :nt])
                nc.vector.tensor_tensor(
                    out=a[:, :nt, 0:D], in0=acc2[:, t0 : t0 + nt, 0:D],
                    in1=rr[:, :nt].to_broadcast([P, nt, D]),
                    op=mybir.AluOpType.mult,
                )
                nc.vector.memset(a[:, :nt, D : D + 1], 1.0)
                o = fpool.tile([P, 4, D], F32, tag="fo")
                for i in range(nt):
                    tp = fppool.tile([D + 1, P], F32, tag="tp", space="PSUM")
                    nc.tensor.transpose(out=tp[:], in_=a[:, i, :],
                                        identity=ident[:])
                    lhs = fpool.tile([D + 1, P], F32, tag="lhs")
                    nc.vector.tensor_copy(out=lhs[:], in_=tp[:])
                    y = fppool.tile([P, D], F32, tag="y", space="PSUM")
                    nc.tensor.matmul(out=y[:], lhsT=lhs[:], rhs=wbs[:],
                                     start=True, stop=True)
                    nc.scalar.copy(out=o[:, i, :], in_=y[:])
                nc.sync.dma_start(out=out_v[:, t0 : t0 + nt, :], in_=o[:, :nt, :])

    nc.compile()
    return nc


def _wrap16(a: np.ndarray) -> np.ndarray:
    w = a.reshape(-1, 16).T
    return np.ascontiguousarray(np.tile(w, (8, 1)))


def _pack_core(rows, cols, nwin):
    """Pack one core's edges into the (window, group) cell schedule.

    Returns (win_rows [nwin, W] int32 node-local row ids (-1 junk),
             tok_col int32 [ntok] table-local col (0 for junk),
             tok_rowloc [ntok] float (window-local row slot, 100 junk))
    or None if packing failed.
    """
    grp = cols // GRP
    deg = np.zeros((NPC, NGRP), np.int64)
    np.add.at(deg, (rows, grp), 1)
    order = np.argsort(-deg.max(1), kind="stable")
    loads = np.zeros((nwin, NGRP), np.int64)
    counts = np.zeros(nwin, np.int64)
    assign = np.full(NPC, -1, np.int64)
    slot_of = np.full(NPC, -1, np.int64)
    # greedy best-fit: place heaviest rows first into the most-loaded
    # window that still fits (tightest fit packs best)
    for r in order:
        d = deg[r]
        ok = (counts < W) & np.all(loads + d <= CELL, axis=1)
        if not ok.any():
            return None
        cand = np.where(ok)[0]
        pick = cand[np.argmin((loads[cand] + d).max(1))]
        assign[r] = pick
        slot_of[r] = counts[pick]
        loads[pick] += d
        counts[pick] += 1

    win_rows = np.full((nwin, W), -1, np.int64)
    win_rows[assign, slot_of] = np.arange(NPC)

    ntokg = nwin * 2 * P
    tok_col = np.zeros(nwin * 2 * P * NGRP, np.int64)
    tok_rowloc = np.full(nwin * 2 * P * NGRP, 100.0, np.float32)
    # order edges by (grp, window, arbitrary); place into cell token ranges
    ewin = assign[rows]
    eslot = slot_of[rows]
    o = np.lexsort((ewin, grp))
    # cell base position for each edge: cumulative within (grp, window)
    gs, ws = grp[o], ewin[o]
    new = np.r_[True, (gs[1:] != gs[:-1]) | (ws[1:] != ws[:-1])]
    gid = np.cumsum(new) - 1
    first = np.arange(len(o))[new]
    rank = np.arange(len(o)) - first[gid]
    pos = gs * ntokg + ws * (2 * P) + rank
    tok_col[pos] = cols[o] % GRP
    tok_rowloc[pos] = eslot[o]
    return win_rows, tok_col, tok_rowloc


def shard_inputs(x, z, edge_index, nwin=210):
    z = np.asarray(z, np.float32)
    x = np.asarray(x, np.float32)
    zn = np.maximum(np.sqrt((z * z).sum(1)), 1e-9)
    zh = z / zn[:, None]
    tab = np.ascontiguousarray(
        np.concatenate([zh, x], axis=1).astype(ml_dtypes.bfloat16))

    row = np.asarray(edge_index[0], np.int64)
    col = np.asarray(edge_index[1], np.int64)
    core = row // NPC

    iota = np.tile(np.arange(W, dtype=np.float32), (P, 1)).astype(
        ml_dtypes.bfloat16)

    in_maps = []
    outmaps = []
    for k in range(N_CORES):
        m = core == k
        packed = _pack_core(row[m] % NPC, col[m], nwin)
        if packed is None:
            return None, None, None
        win_rows, tok_col, tok_rowloc = packed
        zhw = np.zeros((D, nwin, W), np.float32)
        valid = win_rows >= 0
        zhw[:, valid] = zh[k * NPC + win_rows[valid]].T
        ntok = tok_col.shape[0]
        in_maps.append({
            **{f"tab{g}": np.ascontiguousarray(tab[g * GRP : (g + 1) * GRP])
               for g in range(NGRP)},
            "zhw": zhw.astype(ml_dtypes.bfloat16),
            "cidx": _wrap16(tok_col.astype(np.int16)),
            "rowloc": np.ascontiguousarray(
                tok_rowloc.reshape(-1, P).T.astype(ml_dtypes.bfloat16)),
            "iota": iota,
        })
        outmaps.append(win_rows)
    return in_maps, outmaps, nwin


def run(x, edge_index, z, Wm, b, alpha, bias_edge, trace=False):
    from concourse.bass_utils import run_bass_kernel_spmd

    nwin = 210
    while True:
        in_maps, outmaps, nwin_used = shard_inputs(x, z, edge_index, nwin)
        if in_maps is not None:
            break
        nwin += 7
    cfg = Cfg(nwin=nwin_used)
    wb = np.ascontiguousarray(
        np.concatenate(
            [np.asarray(Wm, np.float32).T, np.asarray(b, np.float32)[None, :]],
            axis=0,
        )
    )
    for m in in_maps:
        m["wb"] = wb
    nc = build_program(cfg, float(np.asarray(alpha)))
    core_ids = list(range(N_CORES))
    res = run_bass_kernel_spmd(nc, in_maps, core_ids, trace=trace)
    out = assemble_output(res.results, outmaps, cfg)
    return out, res


def assemble_output(results, outmaps, cfg):
    out = np.zeros((N_NODES, D), np.float32)
    for k in range(N_CORES):
        o = np.asarray(results[k]["out"], np.float32)  # [acc_rows, D]
        win_rows = outmaps[k]
        # device row (w, r) interleaved: acc2 put even windows on parts
        # 0:64, odd on 64:128; tile t covers windows (2t, 2t+1);
        # out row index = t*128 + (w%2)*64 + r
        w_idx = np.repeat(np.arange(cfg.nwin), W)
        r_idx = np.tile(np.arange(W), cfg.nwin)
        dev_pos = (w_idx // 2) * P + (w_idx % 2) * W + r_idx
        flat = win_rows.reshape(-1)
        valid = flat >= 0
        out[k * NPC + flat[valid]] = o[dev_pos[valid]]
    return out


def kernel(**inputs) -> np.ndarray:
    out, _ = run(
        inputs["x"], inputs["edge_index"], inputs["z"],
        inputs["W"], inputs["b"], inputs["alpha"], inputs["bias_edge"],
    )
    return out


# revision 6
# speedup vs baseline: 2.7652x; 1.3796x over previous
"""Trainium2 Bass kernel for nn_DiracGraphConv (GNN edge-softmax message passing).

V4 design (8 NeuronCores, SPMD, no collectives):
  - Shard edges by destination-node slab: core k owns rows
    [k*12500, (k+1)*12500). Per-core output slabs are disjoint; the full
    output is a host-side concatenation (row un-permute).
  - ONE SWDGE descriptor per edge (the previous kernel used three):
    a transposed dma_gather of 256B rows from a bf16 [zh | x] node table
    (zh = z/||z|| precomputed host-side), giving zh_col^T / x_col^T
    directly in SBUF. gpsimd descriptor generation (~7ns/desc, the hard
    bottleneck of this problem) is therefore minimized.
  - The segment-sum (edge softmax numerator/denominator) is done on the
    PE with a mask trick instead of dma_scatter_add:
      * rows are packed into NWIN windows of 64 rows (host bin-packing,
        balanced so every (window, col-group) cell holds <= 256 edges =
        2 subchunks of 128 -- a compile-time uniform schedule).
      * M2[e, r] = zh_col[e] . zh_win[r] via matmul (lhsT = gathered
        zh^T slice, rhs = per-window zh_win^T kept resident in SBUF).
      * masked[e, r] = (rowloc[e] == r) * exp(alpha*M2 - |alpha|); the
        exp shift is valid by softmax shift-invariance.
      * PSUM [64, 65] += masked^T @ [x_col | 1] accumulates messages and
        denominator per window; drained into an SBUF accumulator.
  - x_col in natural layout comes from dma_start_transpose (XBAR maps
    token t -> (partition t%128, slot t//128), matching the gather).
  - Final phase: out = (msgs/(denom+eps)) @ W^T + b via PE transpose +
    matmul with [W^T; b], from SBUF, then one DMA out per core.
"""

import sys

sys.path.insert(0, "/opt/trn_rl_repo")

from dataclasses import dataclass

import numpy as np
import ml_dtypes

from concourse import bacc, bass, mybir, tile
from concourse.library_config import mlp as MLP_LIB
from concourse.masks import make_identity

P = 128
F32 = mybir.dt.float32
BF16 = mybir.dt.bfloat16
I16 = mybir.dt.int16
EPS_DENOM = 1e-9

N_NODES = 100000
N_EDGES = 1600000
D = 64
DD = 128           # [zh | x] row elems (bf16) = 256B
N_CORES = 8
NPC = 12500        # nodes per core
NGRP = 4           # col groups (int16 gather index limit)
GRP = 25000        # nodes per col group
W = 64             # rows per window
CELL = 256         # max edges per (window, group) cell = 2 subchunks
STILE = 14         # slots per super-tile (2 gather instrs of 896)
GB = 896           # tokens per gather instruction


@dataclass(frozen=True)
class Cfg:
    nwin: int = 210  # windows per core; nwin % 7 == 0

    @property
    def nslotg(self) -> int:
        return self.nwin * 2          # subchunk slots per group stream

    @property
    def ntokg(self) -> int:
        return self.nslotg * P        # tokens per group stream

    @property
    def ntok(self) -> int:
        return self.ntokg * NGRP

    @property
    def acc_rows(self) -> int:
        return self.nwin * W


def build_program(cfg: Cfg, alpha: float):
    nwin = cfg.nwin
    nslotg = cfg.nslotg
    n_stile = nslotg // STILE

    nc = bacc.Bacc(
        "TRN2", target_bir_lowering=False, debug=False, num_swdge_queues=2
    )

    tabs = [
        nc.dram_tensor(f"tab{g}", [GRP, DD], BF16, kind="ExternalInput").ap()
        for g in range(NGRP)
    ]
    zhw = nc.dram_tensor("zhw", [D, nwin, W], BF16, kind="ExternalInput").ap()
    cidx = nc.dram_tensor(
        "cidx", [P, cfg.ntok // 16], I16, kind="ExternalInput"
    ).ap()
    rowloc = nc.dram_tensor(
        "rowloc", [P, cfg.ntok // P], BF16, kind="ExternalInput"
    ).ap()
    iota = nc.dram_tensor("iota", [P, W], BF16, kind="ExternalInput").ap()
    wb = nc.dram_tensor("wb", [D + 1, D], F32, kind="ExternalInput").ap()
    out = nc.dram_tensor("out", [cfg.acc_rows, D], F32, kind="ExternalOutput").ap()

    with tile.TileContext(nc) as tc:
        with (
            tc.tile_pool(name="const", bufs=1) as cpool,
            tc.tile_pool(name="gath", bufs=3) as gpool,
            tc.tile_pool(name="work", bufs=3) as wpool,
            tc.tile_pool(name="fin", bufs=2) as fpool,
            tc.tile_pool(name="mpsum", bufs=2, space="PSUM") as mpool,
            tc.tile_pool(name="cpsum", bufs=2, space="PSUM") as clpool,
            tc.tile_pool(name="fpsum", bufs=2, space="PSUM") as fppool,
        ):
            nc.gpsimd.load_library(MLP_LIB)
            # ---- resident SBUF state ----
            zhwt = cpool.tile([D, nwin, W], BF16, tag="zhwt")
            nc.sync.dma_start(out=zhwt[:], in_=zhw[:, :, :])
            cix = cpool.tile([P, cfg.ntok // 16], I16, tag="cix")
            nc.sync.dma_start(out=cix[:], in_=cidx[:, :])
            rlt = cpool.tile([P, cfg.ntok // P], BF16, tag="rlt")
            nc.sync.dma_start(out=rlt[:], in_=rowloc[:, :])
            iot = cpool.tile([P, 1, W], BF16, tag="iot")
            nc.sync.dma_start(out=iot[:, 0, :], in_=iota[:, :])
            wbs = cpool.tile([D + 1, D], F32, tag="wbs")
            nc.sync.dma_start(out=wbs[:], in_=wb[:, :])
            ident = cpool.tile([P, P], F32, tag="ident")
            make_identity(nc, ident[:])
            cb = cpool.tile([P, 1], F32, tag="cb")
            nc.vector.memset(cb[:], -abs(float(alpha)))
            acc = cpool.tile([D, nwin, D + 1], F32, tag="acc")
            nc.vector.memset(acc[:], 0.0)

            # ---- edge phase ----
            for g in range(NGRP):
                tab_g = tabs[g][:, :]
                for st in range(n_stile):
                    s0 = st * STILE                     # slot in group stream
                    tok0 = g * cfg.ntokg + (s0 * P)     # global token base
                    # two 896-token transposed gathers -> [128, 1, 1792]
                    gt = gpool.tile([P, 1, STILE * P], BF16, tag="gt")
                    for h in range(2):
                        nc.gpsimd.dma_gather(
                            gt[:, :, h * GB : (h + 1) * GB],
                            tab_g,
                            cix[:, (tok0 + h * GB) // 16 : (tok0 + (h + 1) * GB) // 16],
                            GB, GB, DD, transpose=True, queue_num=h,
                        )
                    # x natural: [64, 1792] -> [128, 14, 64]
                    xn = wpool.tile([P, STILE, D], BF16, tag="xn")
                    nc.sync.dma_start_transpose(out=xn[:], in_=gt[64:128, 0, :])
                    # aug copy [x | 1]
                    xa = wpool.tile([P, STILE, D + 1], BF16, tag="xa")
                    nc.vector.memset(xa[:, :, D : D + 1], 1.0)
                    nc.scalar.copy(out=xa[:, :, 0:D], in_=xn[:])
                    # row-eq mask for the super-tile
                    rl = rlt[:, tok0 // P : tok0 // P + STILE]
                    eqm = wpool.tile([P, STILE, W], BF16, tag="eqm")
                    nc.vector.tensor_tensor(
                        out=eqm[:],
                        in0=rl.to_broadcast([P, STILE, W]),
                        in1=iot[:].to_broadcast([P, STILE, W]),
                        op=mybir.AluOpType.is_equal,
                    )
                    # M2 per subchunk; PSUM halves (7 slots = 1792B/bank)
                    ex = wpool.tile([P, STILE, W], BF16, tag="ex")
                    for h in range(2):
                        m2 = mpool.tile([P, STILE // 2, W], F32, tag="m2",
                                        space="PSUM")
                        for si in range(STILE // 2):
                            s = h * (STILE // 2) + si
                            w_id = (s0 + s) // 2
                            nc.tensor.matmul(
                                out=m2[:, si, :],
                                lhsT=gt[0:D, 0, s * P : (s + 1) * P],
                                rhs=zhwt[:, w_id, :],
                                start=True, stop=True,
                            )
                        nc.scalar.activation(
                            out=ex[:, h * (STILE // 2) : (h + 1) * (STILE // 2), :],
                            in_=m2[:],
                            func=mybir.ActivationFunctionType.Exp,
                            bias=cb[:], scale=float(alpha),
                        )
                    nc.vector.tensor_tensor(
                        out=ex[:], in0=ex[:], in1=eqm[:],
                        op=mybir.AluOpType.mult,
                    )
                    # aggregate: 7 cells x 2 subchunks
                    for c in range(STILE // 2):
                        w_id = (s0 + 2 * c) // 2
                        cell = clpool.tile([W, D + 1], F32, tag="cell",
                                           space="PSUM")
                        for j in range(2):
                            s = 2 * c + j
                            nc.tensor.matmul(
                                out=cell[:],
                                lhsT=ex[:, s, :],
                                rhs=xa[:, s, :],
                                start=(j == 0), stop=(j == 1),
                            )
                        nc.vector.tensor_tensor(
                            out=acc[:, w_id, :], in0=acc[:, w_id, :],
                            in1=cell[:], op=mybir.AluOpType.add,
                        )

            # ---- final phase: out = (msgs/(denom+eps)) @ W^T + b ----
            # repack [64, nwin, 65] -> [128, nwin//2, 65] via SBUF->SBUF DMA
            acc2 = cpool.tile([P, nwin // 2, D + 1], F32, tag="acc2")
            nc.sync.dma_start(out=acc2[0:D, :, :], in_=acc[:, 0::2, :])
            nc.sync.dma_start(out=acc2[D:P, :, :], in_=acc[:, 1::2, :])
            out_v = out.rearrange("(t p) d -> p t d", p=P)
            nt_all = nwin // 2
            for t0 in range(0, nt_all, 4):
                nt = min(4, nt_all - t0)
                a = fpool.tile([P, 4, D + 1], F32, tag="fa")
                dplus = fpool.tile([P, 4], F32, tag="dplus")
                nc.vector.tensor_scalar_add(
                    dplus[:, :nt], acc2[:, t0 : t0 + nt, D : D + 1], EPS_DENOM
                )
                rr = fpool.tile([P, 4], F32, tag="rr")
                nc.vector.reciprocal(out=rr[:, :nt], in_=dplus[:, :nt])
                nc.vector.tensor_tensor(
                    out=a[:, :nt, 0:D], in0=acc2[:, t0 : t0 + nt, 0:D],
                    in1=rr[:, :nt].to_broadcast([P, nt, D]),
                    op=mybir.AluOpType.mult,
                )
                nc.vector.memset(a[:, :nt, D : D + 1], 1.0)
                o = fpool.tile([P, 4, D], F32, tag="fo")
                for i in range(nt):
                    tp = fppool.tile([D + 1, P], F32, tag="tp", space="PSUM")
                    nc.tensor.transpose(out=tp[:], in_=a[:, i, :],
                                        identity=ident[:])
                    lhs = fpool.tile([D + 1, P], F32, tag="lhs")
                    nc.vector.tensor_copy(out=lhs[:], in_=tp[:])
                    y = fppool.tile([P, D], F32, tag="y", space="PSUM")
                    nc.tensor.matmul(out=y[:], lhsT=lhs[:], rhs=wbs[:],
                                     start=True, stop=True)
                    nc.scalar.copy(out=o[:, i, :], in_=y[:])
                nc.sync.dma_start(out=out_v[:, t0 : t0 + nt, :], in_=o[:, :nt, :])

    nc.compile()
    return nc


def _wrap16(a: np.ndarray) -> np.ndarray:
    w = a.reshape(-1, 16).T
    return np.ascontiguousarray(np.tile(w, (8, 1)))


def _pack_core(rows, cols, nwin):
    """Pack one core's edges into the (window, group) cell schedule.

    Returns (win_rows [nwin, W] int32 node-local row ids (-1 junk),
             tok_col int32 [ntok] table-local col (0 for junk),
             tok_rowloc [ntok] float (window-local row slot, 100 junk))
    or None if packing failed.
    """
    grp = cols // GRP
    deg = np.zeros((NPC, NGRP), np.int64)
    np.add.at(deg, (rows, grp), 1)
    order = np.argsort(-deg.max(1), kind="stable")
    loads = np.zeros((nwin, NGRP), np.int64)
    counts = np.zeros(nwin, np.int64)
    assign = np.full(NPC, -1, np.int64)
    slot_of = np.full(NPC, -1, np.int64)
    # greedy best-fit: place heaviest rows first into the most-loaded
    # window that still fits (tightest fit packs best)
    for r in order:
        d = deg[r]
        ok = (counts < W) & np.all(loads + d <= CELL, axis=1)
        if not ok.any():
            return None
        cand = np.where(ok)[0]
        pick = cand[np.argmin((loads[cand] + d).max(1))]
        assign[r] = pick
        slot_of[r] = counts[pick]
        loads[pick] += d
        counts[pick] += 1

    win_rows = np.full((nwin, W), -1, np.int64)
    win_rows[assign, slot_of] = np.arange(NPC)

    ntokg = nwin * 2 * P
    tok_col = np.zeros(nwin * 2 * P * NGRP, np.int64)
    tok_rowloc = np.full(nwin * 2 * P * NGRP, 100.0, np.float32)
    # order edges by (grp, window, arbitrary); place into cell token ranges
    ewin = assign[rows]
    eslot = slot_of[rows]
    o = np.lexsort((ewin, grp))
    # cell base position for each edge: cumulative within (grp, window)
    gs, ws = grp[o], ewin[o]
    new = np.r_[True, (gs[1:] != gs[:-1]) | (ws[1:] != ws[:-1])]
    gid = np.cumsum(new) - 1
    first = np.arange(len(o))[new]
    rank = np.arange(len(o)) - first[gid]
    pos = gs * ntokg + ws * (2 * P) + rank
    tok_col[pos] = cols[o] % GRP
    tok_rowloc[pos] = eslot[o]
    return win_rows, tok_col, tok_rowloc


def shard_inputs(x, z, edge_index, nwin=210):
    z = np.asarray(z, np.float32)
    x = np.asarray(x, np.float32)
    zn = np.maximum(np.sqrt((z * z).sum(1)), 1e-9)
    zh = z / zn[:, None]
    tab = np.ascontiguousarray(
        np.concatenate([zh, x], axis=1).astype(ml_dtypes.bfloat16))

    row = np.asarray(edge_index[0], np.int64)
    col = np.asarray(edge_index[1], np.int64)
    core = row // NPC

    iota = np.tile(np.arange(W, dtype=np.float32), (P, 1)).astype(
        ml_dtypes.bfloat16)

    in_maps = []
    outmaps = []
    for k in range(N_CORES):
        m = core == k
        packed = _pack_core(row[m] % NPC, col[m], nwin)
        if packed is None:
            return None, None, None
        win_rows, tok_col, tok_rowloc = packed
        zhw = np.zeros((D, nwin, W), np.float32)
        valid = win_rows >= 0
        zhw[:, valid] = zh[k * NPC + win_rows[valid]].T
        ntok = tok_col.shape[0]
        in_maps.append({
            **{f"tab{g}": np.ascontiguousarray(tab[g * GRP : (g + 1) * GRP])
               for g in range(NGRP)},
            "zhw": zhw.astype(ml_dtypes.bfloat16),
            "cidx": _wrap16(tok_col.astype(np.int16)),
            "rowloc": np.ascontiguousarray(
                tok_rowloc.reshape(-1, P).T.astype(ml_dtypes.bfloat16)),
            "iota": iota,
        })
        outmaps.append(win_rows)
    return in_maps, outmaps, nwin


def run(x, edge_index, z, Wm, b, alpha, bias_edge, trace=False):
    from concourse.bass_utils import run_bass_kernel_spmd

    nwin = 210
    while True:
        in_maps, outmaps, nwin_used = shard_inputs(x, z, edge_index, nwin)
        if in_maps is not None:
            break
        nwin += 7
    cfg = Cfg(nwin=nwin_used)
    wb = np.ascontiguousarray(
        np.concatenate(
            [np.asarray(Wm, np.float32).T, np.asarray(b, np.float32)[None, :]],
            axis=0,
        )
    )
    for m in in_maps:
        m["wb"] = wb
    nc = build_program(cfg, float(np.asarray(alpha)))
    core_ids = list(range(N_CORES))
    res = run_bass_kernel_spmd(nc, in_maps, core_ids, trace=trace)
    out = assemble_output(res.results, outmaps, cfg)
    return out, res


def assemble_output(results, outmaps, cfg):
    out = np.zeros((N_NODES, D), np.float32)
    for k in range(N_CORES):
        o = np.asarray(results[k]["out"], np.float32)  # [acc_rows, D]
        win_rows = outmaps[k]
        # device row (w, r) interleaved: acc2 put even windows on parts
        # 0:64, odd on 64:128; tile t covers windows (2t, 2t+1);
        # out row index = t*128 + (w%2)*64 + r
        w_idx = np.repeat(np.arange(cfg.nwin), W)
        r_idx = np.tile(np.arange(W), cfg.nwin)
        dev_pos = (w_idx // 2) * P + (w_idx % 2) * W + r_idx
        flat = win_rows.reshape(-1)
        valid = flat >= 0
        out[k * NPC + flat[valid]] = o[dev_pos[valid]]
    return out


def kernel(**inputs) -> np.ndarray:
    out, _ = run(
        inputs["x"], inputs["edge_index"], inputs["z"],
        inputs["W"], inputs["b"], inputs["alpha"], inputs["bias_edge"],
    )
    return out


# revision 7
# speedup vs baseline: 2.8093x; 1.0160x over previous
"""Trainium2 Bass kernel for nn_DiracGraphConv (GNN edge-softmax message passing).

V4 design (8 NeuronCores, SPMD, no collectives):
  - Shard edges by destination-node slab: core k owns rows
    [k*12500, (k+1)*12500). Per-core output slabs are disjoint; the full
    output is a host-side concatenation (row un-permute).
  - ONE SWDGE descriptor per edge (the previous kernel used three):
    a transposed dma_gather of 256B rows from a bf16 [zh | x] node table
    (zh = z/||z|| precomputed host-side), giving zh_col^T / x_col^T
    directly in SBUF. gpsimd descriptor generation (~7ns/desc, the hard
    bottleneck of this problem) is therefore minimized.
  - The segment-sum (edge softmax numerator/denominator) is done on the
    PE with a mask trick instead of dma_scatter_add:
      * rows are packed into NWIN windows of 64 rows (host bin-packing,
        balanced so every (window, col-group) cell holds <= 256 edges =
        2 subchunks of 128 -- a compile-time uniform schedule).
      * M2[e, r] = zh_col[e] . zh_win[r] via matmul (lhsT = gathered
        zh^T slice, rhs = per-window zh_win^T kept resident in SBUF).
      * masked[e, r] = (rowloc[e] == r) * exp(alpha*M2 - |alpha|); the
        exp shift is valid by softmax shift-invariance.
      * PSUM [64, 65] += masked^T @ [x_col | 1] accumulates messages and
        denominator per window; drained into an SBUF accumulator.
  - x_col in natural layout comes from dma_start_transpose (XBAR maps
    token t -> (partition t%128, slot t//128), matching the gather).
  - Final phase: out = (msgs/(denom+eps)) @ W^T + b via PE transpose +
    matmul with [W^T; b], from SBUF, then one DMA out per core.
"""

import sys

sys.path.insert(0, "/opt/trn_rl_repo")

from dataclasses import dataclass

import numpy as np
import ml_dtypes

from concourse import bacc, bass, mybir, tile
from concourse.library_config import mlp as MLP_LIB
from concourse.masks import make_identity

P = 128
F32 = mybir.dt.float32
BF16 = mybir.dt.bfloat16
I16 = mybir.dt.int16
EPS_DENOM = 1e-9

N_NODES = 100000
N_EDGES = 1600000
D = 64
DD = 128           # [zh | x] row elems (bf16) = 256B
N_CORES = 8
NPC = 12500        # nodes per core
NGRP = 4           # col groups (int16 gather index limit)
GRP = 25000        # nodes per col group
W = 64             # rows per window
CELL = 256         # max edges per (window, group) cell = 2 subchunks
STILE = 14         # slots per super-tile (2 gather instrs of 896)
GB = 896           # tokens per gather instruction


@dataclass(frozen=True)
class Cfg:
    nwin: int = 210  # windows per core; nwin % 7 == 0

    @property
    def nslotg(self) -> int:
        return self.nwin * 2          # subchunk slots per group stream

    @property
    def ntokg(self) -> int:
        return self.nslotg * P        # tokens per group stream

    @property
    def ntok(self) -> int:
        return self.ntokg * NGRP

    @property
    def acc_rows(self) -> int:
        return self.nwin * W


def build_program(cfg: Cfg, alpha: float):
    nwin = cfg.nwin
    nslotg = cfg.nslotg
    n_stile = nslotg // STILE

    nc = bacc.Bacc(
        "TRN2", target_bir_lowering=False, debug=False, num_swdge_queues=4
    )

    tabs = [
        nc.dram_tensor(f"tab{g}", [GRP, DD], BF16, kind="ExternalInput").ap()
        for g in range(NGRP)
    ]
    zhw = nc.dram_tensor("zhw", [D, nwin, W], BF16, kind="ExternalInput").ap()
    cidx = nc.dram_tensor(
        "cidx", [P, cfg.ntok // 16], I16, kind="ExternalInput"
    ).ap()
    rowloc = nc.dram_tensor(
        "rowloc", [P, cfg.ntok // P], BF16, kind="ExternalInput"
    ).ap()
    iota = nc.dram_tensor("iota", [P, W], BF16, kind="ExternalInput").ap()
    wb = nc.dram_tensor("wb", [D + 1, D], F32, kind="ExternalInput").ap()
    out = nc.dram_tensor("out", [cfg.acc_rows, D], F32, kind="ExternalOutput").ap()

    with tile.TileContext(nc) as tc:
        with (
            tc.tile_pool(name="const", bufs=1) as cpool,
            tc.tile_pool(name="gath", bufs=4) as gpool,
            tc.tile_pool(name="work", bufs=3) as wpool,
            tc.tile_pool(name="fin", bufs=2) as fpool,
            tc.tile_pool(name="mpsum", bufs=2, space="PSUM") as mpool,
            tc.tile_pool(name="cpsum", bufs=2, space="PSUM") as clpool,
            tc.tile_pool(name="fpsum", bufs=2, space="PSUM") as fppool,
        ):
            nc.gpsimd.load_library(MLP_LIB)
            # ---- resident SBUF state ----
            zhwt = cpool.tile([D, nwin, W], BF16, tag="zhwt")
            nc.sync.dma_start(out=zhwt[:], in_=zhw[:, :, :])
            cix = cpool.tile([P, cfg.ntok // 16], I16, tag="cix")
            nc.sync.dma_start(out=cix[:], in_=cidx[:, :])
            rlt = cpool.tile([P, cfg.ntok // P], BF16, tag="rlt")
            nc.sync.dma_start(out=rlt[:], in_=rowloc[:, :])
            iot = cpool.tile([P, 1, W], BF16, tag="iot")
            nc.sync.dma_start(out=iot[:, 0, :], in_=iota[:, :])
            wbs = cpool.tile([D + 1, D], F32, tag="wbs")
            nc.sync.dma_start(out=wbs[:], in_=wb[:, :])
            ident = cpool.tile([P, P], F32, tag="ident")
            make_identity(nc, ident[:])
            cb = cpool.tile([P, 1], F32, tag="cb")
            nc.vector.memset(cb[:], -abs(float(alpha)))
            acc = cpool.tile([D, nwin, D + 1], F32, tag="acc")
            nc.vector.memset(acc[:], 0.0)

            # ---- edge phase ----
            for g in range(NGRP):
                tab_g = tabs[g][:, :]
                for st in range(n_stile):
                    s0 = st * STILE                     # slot in group stream
                    tok0 = g * cfg.ntokg + (s0 * P)     # global token base
                    # two 896-token transposed gathers -> [128, 1, 1792]
                    gt = gpool.tile([P, 1, STILE * P], BF16, tag="gt")
                    for h in range(2):
                        nc.gpsimd.dma_gather(
                            gt[:, :, h * GB : (h + 1) * GB],
                            tab_g,
                            cix[:, (tok0 + h * GB) // 16 : (tok0 + (h + 1) * GB) // 16],
                            GB, GB, DD, transpose=True,
                            queue_num=(st * 2 + h) % 4,
                        )
                    # x natural: [64, 1792] -> [128, 14, 64]
                    xn = wpool.tile([P, STILE, D], BF16, tag="xn")
                    nc.sync.dma_start_transpose(out=xn[:], in_=gt[64:128, 0, :])
                    # aug copy [x | 1]
                    xa = wpool.tile([P, STILE, D + 1], BF16, tag="xa")
                    nc.vector.memset(xa[:, :, D : D + 1], 1.0)
                    nc.scalar.copy(out=xa[:, :, 0:D], in_=xn[:])
                    # row-eq mask for the super-tile
                    rl = rlt[:, tok0 // P : tok0 // P + STILE]
                    eqm = wpool.tile([P, STILE, W], BF16, tag="eqm")
                    nc.vector.tensor_tensor(
                        out=eqm[:],
                        in0=rl.to_broadcast([P, STILE, W]),
                        in1=iot[:].to_broadcast([P, STILE, W]),
                        op=mybir.AluOpType.is_equal,
                    )
                    # M2 per subchunk; PSUM halves (7 slots = 1792B/bank)
                    ex = wpool.tile([P, STILE, W], BF16, tag="ex")
                    for h in range(2):
                        m2 = mpool.tile([P, STILE // 2, W], F32, tag="m2",
                                        space="PSUM")
                        for si in range(STILE // 2):
                            s = h * (STILE // 2) + si
                            w_id = (s0 + s) // 2
                            nc.tensor.matmul(
                                out=m2[:, si, :],
                                lhsT=gt[0:D, 0, s * P : (s + 1) * P],
                                rhs=zhwt[:, w_id, :],
                                start=True, stop=True,
                            )
                        nc.scalar.activation(
                            out=ex[:, h * (STILE // 2) : (h + 1) * (STILE // 2), :],
                            in_=m2[:],
                            func=mybir.ActivationFunctionType.Exp,
                            bias=cb[:], scale=float(alpha),
                        )
                    nc.vector.tensor_tensor(
                        out=ex[:], in0=ex[:], in1=eqm[:],
                        op=mybir.AluOpType.mult,
                    )
                    # aggregate: 7 cells x 2 subchunks
                    for c in range(STILE // 2):
                        w_id = (s0 + 2 * c) // 2
                        cell = clpool.tile([W, D + 1], F32, tag="cell",
                                           space="PSUM")
                        for j in range(2):
                            s = 2 * c + j
                            nc.tensor.matmul(
                                out=cell[:],
                                lhsT=ex[:, s, :],
                                rhs=xa[:, s, :],
                                start=(j == 0), stop=(j == 1),
                            )
                        nc.vector.tensor_tensor(
                            out=acc[:, w_id, :], in0=acc[:, w_id, :],
                            in1=cell[:], op=mybir.AluOpType.add,
                        )

            # ---- final phase: out = (msgs/(denom+eps)) @ W^T + b ----
            # repack [64, nwin, 65] -> [128, nwin//2, 65] via SBUF->SBUF DMA
            acc2 = cpool.tile([P, nwin // 2, D + 1], F32, tag="acc2")
            nc.sync.dma_start(out=acc2[0:D, :, :], in_=acc[:, 0::2, :])
            nc.sync.dma_start(out=acc2[D:P, :, :], in_=acc[:, 1::2, :])
            out_v = out.rearrange("(t p) d -> p t d", p=P)
            nt_all = nwin // 2
            for t0 in range(0, nt_all, 4):
                nt = min(4, nt_all - t0)
                a = fpool.tile([P, 4, D + 1], F32, tag="fa")
                dplus = fpool.tile([P, 4], F32, tag="dplus")
                nc.vector.tensor_scalar_add(
                    dplus[:, :nt], acc2[:, t0 : t0 + nt, D : D + 1], EPS_DENOM
                )
                rr = fpool.tile([P, 4], F32, tag="rr")
                nc.vector.reciprocal(out=rr[:, :nt], in_=dplus[:, :nt])
                nc.vector.tensor_tensor(
                    out=a[:, :nt, 0:D], in0=acc2[:, t0 : t0 + nt, 0:D],
                    in1=rr[:, :nt].to_broadcast([P, nt, D]),
                    op=mybir.AluOpType.mult,
                )
                nc.vector.memset(a[:, :nt, D : D + 1], 1.0)
                o = fpool.tile([P, 4, D], F32, tag="fo")
                for i in range(nt):
                    tp = fppool.tile([D + 1, P], F32, tag="tp", space="PSUM")
                    nc.tensor.transpose(out=tp[:], in_=a[:, i, :],
                                        identity=ident[:])
                    lhs = fpool.tile([D + 1, P], F32, tag="lhs")
                    nc.vector.tensor_copy(out=lhs[:], in_=tp[:])
                    y = fppool.tile([P, D], F32, tag="y", space="PSUM")
                    nc.tensor.matmul(out=y[:], lhsT=lhs[:], rhs=wbs[:],
                                     start=True, stop=True)
                    nc.scalar.copy(out=o[:, i, :], in_=y[:])
                nc.sync.dma_start(out=out_v[:, t0 : t0 + nt, :], in_=o[:, :nt, :])

    nc.compile()
    return nc


def _wrap16(a: np.ndarray) -> np.ndarray:
    w = a.reshape(-1, 16).T
    return np.ascontiguousarray(np.tile(w, (8, 1)))


def _pack_core(rows, cols, nwin):
    """Pack one core's edges into the (window, group) cell schedule.

    Returns (win_rows [nwin, W] int32 node-local row ids (-1 junk),
             tok_col int32 [ntok] table-local col (0 for junk),
             tok_rowloc [ntok] float (window-local row slot, 100 junk))
    or None if packing failed.
    """
    grp = cols // GRP
    deg = np.zeros((NPC, NGRP), np.int64)
    np.add.at(deg, (rows, grp), 1)
    order = np.argsort(-deg.max(1), kind="stable")
    loads = np.zeros((nwin, NGRP), np.int64)
    counts = np.zeros(nwin, np.int64)
    assign = np.full(NPC, -1, np.int64)
    slot_of = np.full(NPC, -1, np.int64)
    # greedy best-fit: place heaviest rows first into the most-loaded
    # window that still fits (tightest fit packs best)
    for r in order:
        d = deg[r]
        ok = (counts < W) & np.all(loads + d <= CELL, axis=1)
        if not ok.any():
            return None
        cand = np.where(ok)[0]
        pick = cand[np.argmin((loads[cand] + d).max(1))]
        assign[r] = pick
        slot_of[r] = counts[pick]
        loads[pick] += d
        counts[pick] += 1

    win_rows = np.full((nwin, W), -1, np.int64)
    win_rows[assign, slot_of] = np.arange(NPC)

    ntokg = nwin * 2 * P
    tok_col = np.zeros(nwin * 2 * P * NGRP, np.int64)
    tok_rowloc = np.full(nwin * 2 * P * NGRP, 100.0, np.float32)
    # order edges by (grp, window, arbitrary); place into cell token ranges
    ewin = assign[rows]
    eslot = slot_of[rows]
    o = np.lexsort((ewin, grp))
    # cell base position for each edge: cumulative within (grp, window)
    gs, ws = grp[o], ewin[o]
    new = np.r_[True, (gs[1:] != gs[:-1]) | (ws[1:] != ws[:-1])]
    gid = np.cumsum(new) - 1
    first = np.arange(len(o))[new]
    rank = np.arange(len(o)) - first[gid]
    pos = gs * ntokg + ws * (2 * P) + rank
    tok_col[pos] = cols[o] % GRP
    tok_rowloc[pos] = eslot[o]
    return win_rows, tok_col, tok_rowloc


def shard_inputs(x, z, edge_index, nwin=210):
    z = np.asarray(z, np.float32)
    x = np.asarray(x, np.float32)
    zn = np.maximum(np.sqrt((z * z).sum(1)), 1e-9)
    zh = z / zn[:, None]
    tab = np.ascontiguousarray(
        np.concatenate([zh, x], axis=1).astype(ml_dtypes.bfloat16))

    row = np.asarray(edge_index[0], np.int64)
    col = np.asarray(edge_index[1], np.int64)
    core = row // NPC

    iota = np.tile(np.arange(W, dtype=np.float32), (P, 1)).astype(
        ml_dtypes.bfloat16)

    in_maps = []
    outmaps = []
    for k in range(N_CORES):
        m = core == k
        packed = _pack_core(row[m] % NPC, col[m], nwin)
        if packed is None:
            return None, None, None
        win_rows, tok_col, tok_rowloc = packed
        zhw = np.zeros((D, nwin, W), np.float32)
        valid = win_rows >= 0
        zhw[:, valid] = zh[k * NPC + win_rows[valid]].T
        ntok = tok_col.shape[0]
        in_maps.append({
            **{f"tab{g}": np.ascontiguousarray(tab[g * GRP : (g + 1) * GRP])
               for g in range(NGRP)},
            "zhw": zhw.astype(ml_dtypes.bfloat16),
            "cidx": _wrap16(tok_col.astype(np.int16)),
            "rowloc": np.ascontiguousarray(
                tok_rowloc.reshape(-1, P).T.astype(ml_dtypes.bfloat16)),
            "iota": iota,
        })
        outmaps.append(win_rows)
    return in_maps, outmaps, nwin


def run(x, edge_index, z, Wm, b, alpha, bias_edge, trace=False):
    from concourse.bass_utils import run_bass_kernel_spmd

    nwin = 210
    while True:
        in_maps, outmaps, nwin_used = shard_inputs(x, z, edge_index, nwin)
        if in_maps is not None:
            break
        nwin += 7
    cfg = Cfg(nwin=nwin_used)
    wb = np.ascontiguousarray(
        np.concatenate(
            [np.asarray(Wm, np.float32).T, np.asarray(b, np.float32)[None, :]],
            axis=0,
        )
    )
    for m in in_maps:
        m["wb"] = wb
    nc = build_program(cfg, float(np.asarray(alpha)))
    core_ids = list(range(N_CORES))
    res = run_bass_kernel_spmd(nc, in_maps, core_ids, trace=trace)
    out = assemble_output(res.results, outmaps, cfg)
    return out, res


def assemble_output(results, outmaps, cfg):
    out = np.zeros((N_NODES, D), np.float32)
    for k in range(N_CORES):
        o = np.asarray(results[k]["out"], np.float32)  # [acc_rows, D]
        win_rows = outmaps[k]
        # device row (w, r) interleaved: acc2 put even windows on parts
        # 0:64, odd on 64:128; tile t covers windows (2t, 2t+1);
        # out row index = t*128 + (w%2)*64 + r
        w_idx = np.repeat(np.arange(cfg.nwin), W)
        r_idx = np.tile(np.arange(W), cfg.nwin)
        dev_pos = (w_idx // 2) * P + (w_idx % 2) * W + r_idx
        flat = win_rows.reshape(-1)
        valid = flat >= 0
        out[k * NPC + flat[valid]] = o[dev_pos[valid]]
    return out


def kernel(**inputs) -> np.ndarray:
    out, _ = run(
        inputs["x"], inputs["edge_index"], inputs["z"],
        inputs["W"], inputs["b"], inputs["alpha"], inputs["bias_edge"],
    )
    return out


# revision 8
# speedup vs baseline: 2.8177x; 1.0030x over previous
"""Trainium2 Bass kernel for nn_DiracGraphConv (GNN edge-softmax message passing).

V4 design (8 NeuronCores, SPMD, no collectives):
  - Shard edges by destination-node slab: core k owns rows
    [k*12500, (k+1)*12500). Per-core output slabs are disjoint; the full
    output is a host-side concatenation (row un-permute).
  - ONE SWDGE descriptor per edge (the previous kernel used three):
    a transposed dma_gather of 256B rows from a bf16 [zh | x] node table
    (zh = z/||z|| precomputed host-side), giving zh_col^T / x_col^T
    directly in SBUF. gpsimd descriptor generation (~7ns/desc, the hard
    bottleneck of this problem) is therefore minimized.
  - The segment-sum (edge softmax numerator/denominator) is done on the
    PE with a mask trick instead of dma_scatter_add:
      * rows are packed into NWIN windows of 64 rows (host bin-packing,
        balanced so every (window, col-group) cell holds <= 256 edges =
        2 subchunks of 128 -- a compile-time uniform schedule).
      * M2[e, r] = zh_col[e] . zh_win[r] via matmul (lhsT = gathered
        zh^T slice, rhs = per-window zh_win^T kept resident in SBUF).
      * masked[e, r] = (rowloc[e] == r) * exp(alpha*M2 - |alpha|); the
        exp shift is valid by softmax shift-invariance.
      * PSUM [64, 65] += masked^T @ [x_col | 1] accumulates messages and
        denominator per window; drained into an SBUF accumulator.
  - x_col in natural layout comes from dma_start_transpose (XBAR maps
    token t -> (partition t%128, slot t//128), matching the gather).
  - Final phase: out = (msgs/(denom+eps)) @ W^T + b via PE transpose +
    matmul with [W^T; b], from SBUF, then one DMA out per core.
"""

import sys

sys.path.insert(0, "/opt/trn_rl_repo")

from dataclasses import dataclass

import numpy as np
import ml_dtypes

from concourse import bacc, bass, mybir, tile
from concourse.library_config import mlp as MLP_LIB
from concourse.masks import make_identity

P = 128
F32 = mybir.dt.float32
BF16 = mybir.dt.bfloat16
I16 = mybir.dt.int16
EPS_DENOM = 1e-9

N_NODES = 100000
N_EDGES = 1600000
D = 64
DD = 128           # [zh | x] row elems (bf16) = 256B
N_CORES = 8
NPC = 12500        # nodes per core
NGRP = 4           # col groups (int16 gather index limit)
GRP = 25000        # nodes per col group
W = 64             # rows per window
CELL = 256         # max edges per (window, group) cell = 2 subchunks
STILE = 14         # slots per super-tile (2 gather instrs of 896)
GB = 896           # tokens per gather instruction


@dataclass(frozen=True)
class Cfg:
    nwin: int = 210  # windows per core; nwin % 7 == 0

    @property
    def nslotg(self) -> int:
        return self.nwin * 2          # subchunk slots per group stream

    @property
    def ntokg(self) -> int:
        return self.nslotg * P        # tokens per group stream

    @property
    def ntok(self) -> int:
        return self.ntokg * NGRP

    @property
    def acc_rows(self) -> int:
        return self.nwin * W


def build_program(cfg: Cfg, alpha: float):
    nwin = cfg.nwin
    nslotg = cfg.nslotg
    n_stile = nslotg // STILE

    nc = bacc.Bacc(
        "TRN2", target_bir_lowering=False, debug=False, num_swdge_queues=4
    )

    tabs = [
        nc.dram_tensor(f"tab{g}", [GRP, DD], BF16, kind="ExternalInput").ap()
        for g in range(NGRP)
    ]
    zhw = nc.dram_tensor("zhw", [D, nwin, W], BF16, kind="ExternalInput").ap()
    cidx = nc.dram_tensor(
        "cidx", [P, cfg.ntok // 16], I16, kind="ExternalInput"
    ).ap()
    rowloc = nc.dram_tensor(
        "rowloc", [P, cfg.ntok // P], BF16, kind="ExternalInput"
    ).ap()
    iota = nc.dram_tensor("iota", [P, W], BF16, kind="ExternalInput").ap()
    wb = nc.dram_tensor("wb", [D + 1, D], F32, kind="ExternalInput").ap()
    out = nc.dram_tensor("out", [cfg.acc_rows, D], F32, kind="ExternalOutput").ap()

    with tile.TileContext(nc) as tc:
        with (
            tc.tile_pool(name="const", bufs=1) as cpool,
            tc.tile_pool(name="gath", bufs=4) as gpool,
            tc.tile_pool(name="work", bufs=3) as wpool,
            tc.tile_pool(name="fin", bufs=2) as fpool,
            tc.tile_pool(name="mpsum", bufs=2, space="PSUM") as mpool,
            tc.tile_pool(name="cpsum", bufs=2, space="PSUM") as clpool,
            tc.tile_pool(name="fpsum", bufs=2, space="PSUM") as fppool,
        ):
            nc.gpsimd.load_library(MLP_LIB)
            # ---- resident SBUF state ----
            zhwt = cpool.tile([D, nwin, W], BF16, tag="zhwt")
            nc.sync.dma_start(out=zhwt[:], in_=zhw[:, :, :])
            cix = cpool.tile([P, cfg.ntok // 16], I16, tag="cix")
            nc.sync.dma_start(out=cix[:], in_=cidx[:, :])
            rlt = cpool.tile([P, cfg.ntok // P], BF16, tag="rlt")
            nc.sync.dma_start(out=rlt[:], in_=rowloc[:, :])
            iot = cpool.tile([P, 1, W], BF16, tag="iot")
            nc.sync.dma_start(out=iot[:, 0, :], in_=iota[:, :])
            wbs = cpool.tile([D + 1, D], F32, tag="wbs")
            nc.sync.dma_start(out=wbs[:], in_=wb[:, :])
            ident = cpool.tile([P, P], F32, tag="ident")
            make_identity(nc, ident[:])
            cb = cpool.tile([P, 1], F32, tag="cb")
            nc.vector.memset(cb[:], -abs(float(alpha)))
            acc = cpool.tile([D, nwin, D + 1], F32, tag="acc")
            nc.vector.memset(acc[:], 0.0)

            # ---- edge phase ----
            for g in range(NGRP):
                tab_g = tabs[g][:, :]
                for st in range(n_stile):
                    s0 = st * STILE                     # slot in group stream
                    tok0 = g * cfg.ntokg + (s0 * P)     # global token base
                    # two 896-token transposed gathers -> [128, 1, 1792]
                    gt = gpool.tile([P, 1, STILE * P], BF16, tag="gt")
                    for h in range(2):
                        nc.gpsimd.dma_gather(
                            gt[:, :, h * GB : (h + 1) * GB],
                            tab_g,
                            cix[:, (tok0 + h * GB) // 16 : (tok0 + (h + 1) * GB) // 16],
                            GB, GB, DD, transpose=True,
                            queue_num=(st * 2 + h) % 4,
                        )
                    # x natural: [64, 1792] -> [128, 14, 64]
                    xn = wpool.tile([P, STILE, D], BF16, tag="xn")
                    nc.sync.dma_start_transpose(out=xn[:], in_=gt[64:128, 0, :])
                    # aug copy [x | 1]
                    xa = wpool.tile([P, STILE, D + 1], BF16, tag="xa")
                    nc.vector.memset(xa[:, :, D : D + 1], 1.0)
                    nc.scalar.copy(out=xa[:, :, 0:D], in_=xn[:])
                    # row-eq mask for the super-tile
                    rl = rlt[:, tok0 // P : tok0 // P + STILE]
                    eqm = wpool.tile([P, STILE, W], BF16, tag="eqm")
                    nc.vector.tensor_tensor(
                        out=eqm[:],
                        in0=rl.to_broadcast([P, STILE, W]),
                        in1=iot[:].to_broadcast([P, STILE, W]),
                        op=mybir.AluOpType.is_equal,
                    )
                    # M2 per subchunk; PSUM halves (7 slots = 1792B/bank)
                    ex = wpool.tile([P, STILE, W], BF16, tag="ex")
                    for h in range(2):
                        m2 = mpool.tile([P, STILE // 2, W], F32, tag="m2",
                                        space="PSUM")
                        for si in range(STILE // 2):
                            s = h * (STILE // 2) + si
                            w_id = (s0 + s) // 2
                            nc.tensor.matmul(
                                out=m2[:, si, :],
                                lhsT=gt[0:D, 0, s * P : (s + 1) * P],
                                rhs=zhwt[:, w_id, :],
                                start=True, stop=True,
                            )
                        nc.scalar.activation(
                            out=ex[:, h * (STILE // 2) : (h + 1) * (STILE // 2), :],
                            in_=m2[:],
                            func=mybir.ActivationFunctionType.Exp,
                            bias=cb[:], scale=float(alpha),
                        )
                    nc.vector.tensor_tensor(
                        out=ex[:], in0=ex[:], in1=eqm[:],
                        op=mybir.AluOpType.mult,
                    )
                    # aggregate: 7 cells x 2 subchunks
                    for c in range(STILE // 2):
                        w_id = (s0 + 2 * c) // 2
                        cell = clpool.tile([W, D + 1], F32, tag="cell",
                                           space="PSUM")
                        for j in range(2):
                            s = 2 * c + j
                            nc.tensor.matmul(
                                out=cell[:],
                                lhsT=ex[:, s, :],
                                rhs=xa[:, s, :],
                                start=(j == 0), stop=(j == 1),
                            )
                        nc.vector.tensor_tensor(
                            out=acc[:, w_id, :], in0=acc[:, w_id, :],
                            in1=cell[:], op=mybir.AluOpType.add,
                        )

            # ---- final phase: out = (msgs/(denom+eps)) @ W^T + b ----
            # repack [64, nwin, 65] -> [128, nwin//2, 65] via SBUF->SBUF DMA
            acc2 = cpool.tile([P, nwin // 2, D + 1], F32, tag="acc2")
            nc.sync.dma_start(out=acc2[0:D, :, :], in_=acc[:, 0::2, :])
            nc.sync.dma_start(out=acc2[D:P, :, :], in_=acc[:, 1::2, :])
            out_v = out.rearrange("(t p) d -> p t d", p=P)
            nt_all = nwin // 2
            for t0 in range(0, nt_all, 4):
                nt = min(4, nt_all - t0)
                a = fpool.tile([P, 4, D + 1], F32, tag="fa")
                dplus = fpool.tile([P, 4], F32, tag="dplus")
                nc.vector.tensor_scalar_add(
                    dplus[:, :nt], acc2[:, t0 : t0 + nt, D : D + 1], EPS_DENOM
                )
                rr = fpool.tile([P, 4], F32, tag="rr")
                nc.vector.reciprocal(out=rr[:, :nt], in_=dplus[:, :nt])
                nc.vector.tensor_tensor(
                    out=a[:, :nt, 0:D], in0=acc2[:, t0 : t0 + nt, 0:D],
                    in1=rr[:, :nt].to_broadcast([P, nt, D]),
                    op=mybir.AluOpType.mult,
                )
                nc.vector.memset(a[:, :nt, D : D + 1], 1.0)
                o = fpool.tile([P, 4, D], F32, tag="fo")
                for i in range(nt):
                    tp = fppool.tile([D + 1, P], F32, tag="tp", space="PSUM")
                    nc.tensor.transpose(out=tp[:], in_=a[:, i, :],
                                        identity=ident[:])
                    lhs = fpool.tile([D + 1, P], F32, tag="lhs")
                    nc.vector.tensor_copy(out=lhs[:], in_=tp[:])
                    y = fppool.tile([P, D], F32, tag="y", space="PSUM")
                    nc.tensor.matmul(out=y[:], lhsT=lhs[:], rhs=wbs[:],
                                     start=True, stop=True)
                    nc.scalar.copy(out=o[:, i, :], in_=y[:])
                nc.sync.dma_start(out=out_v[:, t0 : t0 + nt, :], in_=o[:, :nt, :])

    nc.compile()
    return nc


def _wrap16(a: np.ndarray) -> np.ndarray:
    w = a.reshape(-1, 16).T
    return np.ascontiguousarray(np.tile(w, (8, 1)))


def _pack_core(rows, cols, nwin):
    """Pack one core's edges into the (window, group) cell schedule.

    Returns (win_rows [nwin, W] int32 node-local row ids (-1 junk),
             tok_col int32 [ntok] table-local col (0 for junk),
             tok_rowloc [ntok] float (window-local row slot, 100 junk))
    or None if packing failed.
    """
    grp = cols // GRP
    deg = np.zeros((NPC, NGRP), np.int64)
    np.add.at(deg, (rows, grp), 1)
    order = np.argsort(-deg.max(1), kind="stable")
    loads = np.zeros((nwin, NGRP), np.int64)
    counts = np.zeros(nwin, np.int64)
    assign = np.full(NPC, -1, np.int64)
    slot_of = np.full(NPC, -1, np.int64)
    # greedy best-fit: place heaviest rows first into the most-loaded
    # window that still fits (tightest fit packs best)
    for r in order:
        d = deg[r]
        ok = (counts < W) & np.all(loads + d <= CELL, axis=1)
        if not ok.any():
            return None
        cand = np.where(ok)[0]
        pick = cand[np.argmin((loads[cand] + d).max(1))]
        assign[r] = pick
        slot_of[r] = counts[pick]
        loads[pick] += d
        counts[pick] += 1

    win_rows = np.full((nwin, W), -1, np.int64)
    win_rows[assign, slot_of] = np.arange(NPC)

    ntokg = nwin * 2 * P
    tok_col = np.zeros(nwin * 2 * P * NGRP, np.int64)
    tok_rowloc = np.full(nwin * 2 * P * NGRP, 100.0, np.float32)
    # order edges by (grp, window, arbitrary); place into cell token ranges
    ewin = assign[rows]
    eslot = slot_of[rows]
    # sort by col within each (grp, window) cell: monotone HBM addresses
    o = np.lexsort((cols, ewin, grp))
    # cell base position for each edge: cumulative within (grp, window)
    gs, ws = grp[o], ewin[o]
    new = np.r_[True, (gs[1:] != gs[:-1]) | (ws[1:] != ws[:-1])]
    gid = np.cumsum(new) - 1
    first = np.arange(len(o))[new]
    rank = np.arange(len(o)) - first[gid]
    pos = gs * ntokg + ws * (2 * P) + rank
    tok_col[pos] = cols[o] % GRP
    tok_rowloc[pos] = eslot[o]
    return win_rows, tok_col, tok_rowloc


def shard_inputs(x, z, edge_index, nwin=210):
    z = np.asarray(z, np.float32)
    x = np.asarray(x, np.float32)
    zn = np.maximum(np.sqrt((z * z).sum(1)), 1e-9)
    zh = z / zn[:, None]
    tab = np.ascontiguousarray(
        np.concatenate([zh, x], axis=1).astype(ml_dtypes.bfloat16))

    row = np.asarray(edge_index[0], np.int64)
    col = np.asarray(edge_index[1], np.int64)
    core = row // NPC

    iota = np.tile(np.arange(W, dtype=np.float32), (P, 1)).astype(
        ml_dtypes.bfloat16)

    in_maps = []
    outmaps = []
    for k in range(N_CORES):
        m = core == k
        packed = _pack_core(row[m] % NPC, col[m], nwin)
        if packed is None:
            return None, None, None
        win_rows, tok_col, tok_rowloc = packed
        zhw = np.zeros((D, nwin, W), np.float32)
        valid = win_rows >= 0
        zhw[:, valid] = zh[k * NPC + win_rows[valid]].T
        ntok = tok_col.shape[0]
        in_maps.append({
            **{f"tab{g}": np.ascontiguousarray(tab[g * GRP : (g + 1) * GRP])
               for g in range(NGRP)},
            "zhw": zhw.astype(ml_dtypes.bfloat16),
            "cidx": _wrap16(tok_col.astype(np.int16)),
            "rowloc": np.ascontiguousarray(
                tok_rowloc.reshape(-1, P).T.astype(ml_dtypes.bfloat16)),
            "iota": iota,
        })
        outmaps.append(win_rows)
    return in_maps, outmaps, nwin


def run(x, edge_index, z, Wm, b, alpha, bias_edge, trace=False):
    from concourse.bass_utils import run_bass_kernel_spmd

    nwin = 210
    while True:
        in_maps, outmaps, nwin_used = shard_inputs(x, z, edge_index, nwin)
        if in_maps is not None:
            break
        nwin += 7
    cfg = Cfg(nwin=nwin_used)
    wb = np.ascontiguousarray(
        np.concatenate(
            [np.asarray(Wm, np.float32).T, np.asarray(b, np.float32)[None, :]],
            axis=0,
        )
    )
    for m in in_maps:
        m["wb"] = wb
    nc = build_program(cfg, float(np.asarray(alpha)))
    core_ids = list(range(N_CORES))
    res = run_bass_kernel_spmd(nc, in_maps, core_ids, trace=trace)
    out = assemble_output(res.results, outmaps, cfg)
    return out, res


def assemble_output(results, outmaps, cfg):
    out = np.zeros((N_NODES, D), np.float32)
    for k in range(N_CORES):
        o = np.asarray(results[k]["out"], np.float32)  # [acc_rows, D]
        win_rows = outmaps[k]
        # device row (w, r) interleaved: acc2 put even windows on parts
        # 0:64, odd on 64:128; tile t covers windows (2t, 2t+1);
        # out row index = t*128 + (w%2)*64 + r
        w_idx = np.repeat(np.arange(cfg.nwin), W)
        r_idx = np.tile(np.arange(W), cfg.nwin)
        dev_pos = (w_idx // 2) * P + (w_idx % 2) * W + r_idx
        flat = win_rows.reshape(-1)
        valid = flat >= 0
        out[k * NPC + flat[valid]] = o[dev_pos[valid]]
    return out


def kernel(**inputs) -> np.ndarray:
    out, _ = run(
        inputs["x"], inputs["edge_index"], inputs["z"],
        inputs["W"], inputs["b"], inputs["alpha"], inputs["bias_edge"],
    )
    return out
